# revision 4
# baseline (speedup 1.0000x reference)
"""EnhancedGatedFusion fused kernel for 8 TRN2 NeuronCores — fp8 DoubleRow.

Math (per token row x1, x2 of emb1/emb2; cat = [x1; x2], C = 2D):
  f_g = sigmoid(cat @ Wf[g].T + bf[g])          g = 0..2
  i_g = sigmoid(cat @ Wi[g].T + bi[g])
  u_g = tanh   (cat @ Wu[g].T + bu[g])
  gate_g = f_g * x1 + i_g * u_g
  fused = sum_g softmax(att_w)[g] * gate_g
  o = sigmoid(cat @ Wo.T + bo)
  out = LayerNorm(x1 + o * tanh(fused)) * gamma + beta

The ten GEMMs run in fp8 e4m3 with perf_mode=DoubleRow (two K-subtiles
per matmul, 2 fp8 weights per PE cell -> ~2x bf16 TensorE throughput at
FD=512). Host packing folds scales into the fp8 operands (xT *= SX,
wT *= SW); the descale by 1/(SX*SW) is fused into the per-column bias
add on VectorE. Operand tiles are pre-packed on the host so every DMA
reads one contiguous run per partition:
  xT [ngrp, P, nct, tg]   fp8 : per-group cat^T tiles.
  wT [NQ*ndc, P, nct, dcq] fp8: per-(quantity, D-chunk) weight tiles,
                                tile order q*ndc + dc.
Sharding: data-parallel over tokens (16384/8 = 2048 per core); weights
replicated. Everything after the matmuls is fp32 (stash bf16).
"""

import sys

sys.path.insert(0, "/opt/trn_rl_repo")

import numpy as np
import ml_dtypes

import concourse.bass as bass
import concourse.tile as tile
from concourse import mybir, bacc
from concourse.bass_utils import run_bass_kernel_spmd

P = 128
N_CORES = 8
N_TOK, D_DIM, G_GATES = 16384, 2048, 3
C_DIM = 2 * D_DIM
NQ = 3 * G_GATES + 1  # f/i/u per gate + output gate
LN_EPS = 1e-5
FP8 = ml_dtypes.float8_e4m3  # TRN float8e4 (max normal 240)
SX = 16.0  # emb scale: |x| < 6 -> < 96
SW = 512.0  # weight scale: |w| < 0.12 -> < 62
INV_S = 1.0 / (SX * SW)


def _bcast_ap(vec: bass.AP, parts: int) -> bass.AP:
    """Partition-broadcast a 1-D DRAM vector to [parts, len]."""
    return bass.AP(tensor=vec.tensor, offset=vec.offset, ap=[[0, parts]] + vec.ap)


def build(
    n_cores: int,
    tokc: int,
    cdim: int,
    ddim: int,
    tg: int,
    dcq: int,
    w_soft,
    eps: float,
    reps: int = 1,
):
    """Build the per-core SPMD Bass program."""
    f32 = mybir.dt.float32
    bf = mybir.dt.bfloat16
    fp8 = mybir.dt.float8e4
    add, mult, sub = (
        mybir.AluOpType.add,
        mybir.AluOpType.mult,
        mybir.AluOpType.subtract,
    )
    AF = mybir.ActivationFunctionType
    DR = mybir.MatmulPerfMode.DoubleRow

    nct = cdim // P  # K tiles
    nt = tg // P  # token tiles per group
    ngrp = tokc // tg  # token groups
    ndc = ddim // dcq  # D chunks
    nsub = ddim // 512  # bn_stats subgroups

    nc = bacc.Bacc("TRN2", target_bir_lowering=False, debug=False, num_devices=n_cores)
    xT = nc.dram_tensor("xT", [ngrp, P, nct, tg], fp8, kind="ExternalInput").ap()
    wT = nc.dram_tensor(
        "wT", [NQ * ndc, P, nct, dcq], fp8, kind="ExternalInput"
    ).ap()
    bv = nc.dram_tensor("bv", [NQ * ddim], f32, kind="ExternalInput").ap()
    emb1 = nc.dram_tensor("emb1", [tokc, ddim], f32, kind="ExternalInput").ap()
    gamma = nc.dram_tensor("gamma", [ddim], f32, kind="ExternalInput").ap()
    beta = nc.dram_tensor("beta", [ddim], f32, kind="ExternalInput").ap()
    out = nc.dram_tensor("out", [tokc, ddim], f32, kind="ExternalOutput").ap()

    with tile.TileContext(nc) as tc:
        with (
            tc.tile_pool(name="const", bufs=1) as const,
            tc.tile_pool(name="xg", bufs=2) as xg_pool,
            tc.tile_pool(name="wq", bufs=4) as wq_pool,
            tc.tile_pool(name="bq", bufs=3) as bq_pool,
            tc.tile_pool(name="et", bufs=nt + 1) as et_pool,
            tc.tile_pool(name="fz", bufs=nt + 1) as fz_pool,
            tc.tile_pool(name="sf", bufs=nt) as sf_pool,
            tc.tile_pool(name="si", bufs=nt) as si_pool,
            tc.tile_pool(name="su", bufs=nt) as su_pool,
            tc.tile_pool(name="so", bufs=2) as so_pool,
            tc.tile_pool(name="tmp", bufs=2) as tmp_pool,
            tc.tile_pool(name="xf", bufs=1) as xf_pool,
            tc.tile_pool(name="st", bufs=4) as st_pool,
            tc.tile_pool(name="ps", bufs=8, space="PSUM") as ps_pool,
        ):
            gamma_t = const.tile([P, ddim], f32)
            nc.sync.dma_start(gamma_t[:], _bcast_ap(gamma, P))
            beta_t = const.tile([P, ddim], f32)
            nc.sync.dma_start(beta_t[:], _bcast_ap(beta, P))
            eps_t = const.tile([P, 1], f32)
            nc.vector.memset(eps_t[:], eps)

            for g in [g for _ in range(reps) for g in range(ngrp)]:
                xg = xg_pool.tile([P, nct, tg], fp8)
                nc.sync.dma_start(xg[:], xT[g])
                xf = xf_pool.tile([P, nt, ddim], f32)

                for dc in range(ndc):
                    dsl = slice(dc * dcq, (dc + 1) * dcq)
                    ets = []
                    for t in range(nt):
                        et = et_pool.tile([P, dcq], f32, tag="et")
                        r0 = g * tg + t * P
                        nc.sync.dma_start(et[:], emb1[r0 : r0 + P, dsl])
                        ets.append(et)
                    fz = [
                        fz_pool.tile([P, dcq], f32, tag="fz", name=f"fz{t}")
                        for t in range(nt)
                    ]
                    sfs, sis = None, None

                    for q in range(NQ):
                        col0 = q * ddim + dc * dcq
                        wq = wq_pool.tile([P, nct, dcq], fp8)
                        nc.sync.dma_start(wq[:], wT[q * ndc + dc])
                        bq = bq_pool.tile([P, dcq], f32)
                        nc.sync.dma_start(bq[:], _bcast_ap(bv[col0 : col0 + dcq], P))

                        kind = "o" if q == NQ - 1 else "fiu"[q % 3]
                        gate_i = q // 3
                        pool = {
                            "f": sf_pool,
                            "i": si_pool,
                            "u": su_pool,
                            "o": so_pool,
                        }[kind]
                        func = AF.Tanh if kind == "u" else AF.Sigmoid

                        stash = []
                        for t in range(nt):
                            ps = ps_pool.tile([P, dcq], f32)
                            for ci in range(0, nct, 2):
                                nc.tensor.matmul(
                                    ps[:],
                                    lhsT=xg[:, ci : ci + 2, t * P : (t + 1) * P],
                                    rhs=wq[:, ci : ci + 2, :],
                                    start=(ci == 0),
                                    stop=(ci == nct - 2),
                                    perf_mode=DR,
                                )
                            # descale + bias in one VectorE op, then
                            # activation on ScalarE (bf16 stash)
                            s = pool.tile([P, dcq], bf, tag=kind)
                            nc.vector.scalar_tensor_tensor(
                                out=s[:],
                                in0=ps[:],
                                scalar=INV_S,
                                in1=bq[:],
                                op0=mult,
                                op1=add,
                            )
                            nc.scalar.activation(s[:], s[:], func)
                            stash.append(s)

                        if kind == "f":
                            sfs = stash
                        elif kind == "i":
                            sis = stash
                        elif kind == "u":
                            wgt = float(w_soft[gate_i])
                            for t in range(nt):
                                tA = tmp_pool.tile([P, dcq], f32, tag="tA")
                                nc.vector.tensor_mul(tA[:], sis[t][:], stash[t][:])
                                tB = tmp_pool.tile([P, dcq], f32, tag="tB")
                                nc.vector.tensor_mul(tB[:], sfs[t][:], ets[t][:])
                                nc.vector.tensor_add(tA[:], tA[:], tB[:])
                                if gate_i == 0:
                                    nc.vector.tensor_scalar_mul(fz[t][:], tA[:], wgt)
                                else:
                                    # fz += tA * w_g
                                    nc.vector.scalar_tensor_tensor(
                                        out=fz[t][:],
                                        in0=tA[:],
                                        scalar=wgt,
                                        in1=fz[t][:],
                                        op0=mult,
                                        op1=add,
                                    )
                        else:  # output gate: x = emb1 + o * tanh(fused)
                            for t in range(nt):
                                th = tmp_pool.tile([P, dcq], f32, tag="tA")
                                nc.scalar.activation(th[:], fz[t][:], AF.Tanh)
                                xc = tmp_pool.tile([P, dcq], f32, tag="tB")
                                nc.vector.tensor_mul(xc[:], stash[t][:], th[:])
                                nc.vector.tensor_add(
                                    xf[:, t, dsl], ets[t][:], xc[:]
                                )

                # LayerNorm on the resident group (in place, then DMA out)
                for t in range(nt):
                    xr = xf[:, t, :]
                    stats = st_pool.tile([P, nsub, 6], f32, tag="stats")
                    for s_i in range(nsub):
                        nc.vector.bn_stats(
                            stats[:, s_i, :], xr[:, s_i * 512 : (s_i + 1) * 512]
                        )
                    mv = st_pool.tile([P, 2], f32, tag="mv")
                    nc.vector.bn_aggr(mv[:], stats[:])
                    rstd = st_pool.tile([P, 1], f32, tag="rstd")
                    nc.scalar.activation(
                        rstd[:], mv[:, 1:2], AF.Sqrt, bias=eps_t[:], scale=1.0
                    )
                    nc.vector.reciprocal(rstd[:], rstd[:])
                    nc.vector.tensor_scalar(
                        out=xr[:],
                        in0=xr[:],
                        scalar1=mv[:, 0:1],
                        scalar2=rstd[:],
                        op0=sub,
                        op1=mult,
                    )
                    nc.vector.tensor_mul(xr[:], xr[:], gamma_t[:])
                    nc.vector.tensor_add(xr[:], xr[:], beta_t[:])
                    r0 = g * tg + t * P
                    nc.sync.dma_start(out[r0 : r0 + P, :], xr[:])
    nc.compile()
    return nc


def _prep_host(
    emb1, emb2, Wf, bfv, Wi, biv, Wu, buv, Wo, bov, att_w, tg=512, dcq=512
):
    """Host-side packing: softmax weights, scaled fp8 operand tiles laid
    out so each DMA reads one contiguous run per partition."""
    emb1 = np.asarray(emb1, dtype=np.float32)
    emb2 = np.asarray(emb2, dtype=np.float32)
    aw = np.asarray(att_w, dtype=np.float64)
    aw = np.exp(aw - aw.max())
    w_soft = (aw / aw.sum()).astype(np.float32)

    nct = C_DIM // P
    ndc = D_DIM // dcq
    cols, bcols = [], []
    for gi in range(G_GATES):
        for W, b in ((Wf, bfv), (Wi, biv), (Wu, buv)):
            cols.append(np.asarray(W[gi], dtype=np.float32).T)
            bcols.append(np.asarray(b[gi], dtype=np.float32))
    cols.append(np.asarray(Wo, dtype=np.float32).T)
    bcols.append(np.asarray(bov, dtype=np.float32))
    wTf = (np.concatenate(cols, axis=1) * SW).astype(FP8)  # [C, NQ*D]
    # -> [NQ*ndc, P, nct, dcq], tile index q*ndc + dc
    wT = np.ascontiguousarray(
        wTf.reshape(nct, P, NQ * ndc, dcq).transpose(2, 1, 0, 3)
    )
    bv = np.concatenate(bcols).astype(np.float32)
    xTf = (
        np.ascontiguousarray(np.concatenate([emb1.T, emb2.T], axis=0)) * SX
    ).astype(FP8)  # [C, N]
    return emb1, xTf, wT, bv, w_soft


def _pack_core_x(xTc, tg):
    """[C, tokc] fp8 -> [ngrp, P, nct, tg] with contiguous partition runs."""
    nct = C_DIM // P
    tokc = xTc.shape[1]
    ngrp = tokc // tg
    return np.ascontiguousarray(
        xTc.reshape(nct, P, ngrp, tg).transpose(2, 1, 0, 3)
    )


def kernel(emb1, emb2, Wf, bf, Wi, bi, Wu, bu, Wo, bo, att_w, gamma, beta):
    tg, dcq = 512, 512
    emb1, xTf, wT, bv, w_soft = _prep_host(
        emb1, emb2, Wf, bf, Wi, bi, Wu, bu, Wo, bo, att_w, tg=tg, dcq=dcq
    )
    gamma = np.asarray(gamma, dtype=np.float32)
    beta = np.asarray(beta, dtype=np.float32)
    tokc = N_TOK // N_CORES

    nc = build(
        n_cores=N_CORES,
        tokc=tokc,
        cdim=C_DIM,
        ddim=D_DIM,
        tg=tg,
        dcq=dcq,
        w_soft=w_soft,
        eps=LN_EPS,
    )
    in_maps = []
    for ci in range(N_CORES):
        s = slice(ci * tokc, (ci + 1) * tokc)
        in_maps.append(
            {
                "xT": _pack_core_x(xTf[:, s], tg),
                "wT": wT,
                "bv": bv,
                "emb1": np.ascontiguousarray(emb1[s]),
                "gamma": gamma,
                "beta": beta,
            }
        )
    res = run_bass_kernel_spmd(nc, in_maps, list(range(N_CORES)))
    return np.concatenate(
        [res.results[i]["out"] for i in range(N_CORES)], axis=0
    ).astype(np.float32)
